# revision 5
# baseline (speedup 1.0000x reference)
"""Sparse-attention TRN2 kernel (v2: DMA-transpose + fused norms).

Reference computation (per batch b):
  pf = normalize(x @ W_pf.T); ns = normalize(x @ W_ns.T); v = x @ W_v.T
  G = pf @ pf.T                                (T x T cosine sims)
  M[u, y] = max_{j<5} G[u, start(y)+j]         (sliding window max, clamped)
  S_pf[x, y] = sum_i w_pf[i] * M[start(x)+i, y]  == (W_band @ M)[x, y]
  S_ns[x, y] = sum_t Q[x, t] * (ns_n[t] . ns_n[y])   with
      Q[x, t] = sum_n w_ns[n] * [inxs[x, n] == t]    (host-precomputed)
  L = S_pf + S_ns + mask(radj);  attn = softmax(L, axis=-1);  out = attn @ v

Kernel computes L.T (y on partitions, x free) so softmax normalization and
the attn@v contraction need no transposes of the T x T tensors.

v2 changes vs the 49us baseline:
  - square+row-sum fused into one scalar_tensor_tensor(accum_out=) per
    128-token block (replaces Square acts + the 1.2us TENSOR_REDUCE)
  - the 16 PE transposes + PSUM evacs replaced by 4 SBUF->SBUF XBAR
    DMA transposes (one [128,512] -> [128,4,128] per batch)
  - quake rsqrt chain moved to gpsimd (frees DVE)
  - input DMAs split across the two HWDGE queues (sync+scalar), first
    tiles (Wcat/WbT blob, xT) prioritized so the first matmul starts ~4us
    earlier; identity matrix input dropped entirely
  - out DMA issued once per batch-pair instead of per batch
"""

import sys

sys.path.insert(0, "/opt/trn_rl_repo")

from contextlib import ExitStack

import numpy as np

import concourse.bacc as bacc
import concourse.bass as bass
import concourse.tile as tile
from concourse import mybir
from concourse._compat import with_exitstack

B, T, C = 32, 256, 128
TNEI = 2
TOPK = 4
NEIGH = 2 * TNEI + 1
N_CORES = 8
BPC = B // N_CORES  # batches per core

F32 = mybir.dt.float32
I32 = mybir.dt.int32
BF16 = mybir.dt.bfloat16

Act = mybir.ActivationFunctionType
Alu = mybir.AluOpType

NP_BF16 = mybir.dt.np(BF16)


def _blk128(a2d):
    """(T, T)->(128, 2T): out[p, u*T+x] = a2d[x, u*128+p]."""
    return np.ascontiguousarray(
        a2d.T.reshape(2, 128, T).transpose(1, 0, 2).reshape(128, 2 * T)
    )


def host_weights(W_pf, W_ns, W_v, v_pf, g_pf, v_ns, g_ns):
    """Constant (replicated) tensors, all pure layout/small-vector prep."""
    w_pf = (g_pf[0] * v_pf / np.linalg.norm(v_pf)).astype(np.float32)
    w_ns = (g_ns[0] * v_ns / np.linalg.norm(v_ns)).astype(np.float32)

    # Banded weight matrix: W_band[x, u] = w_pf[u - start(x)] on the band.
    start = np.clip(np.arange(T) - TNEI, 0, T - NEIGH)
    W_band = np.zeros((T, T), np.float32)
    for i in range(NEIGH):
        W_band[np.arange(T), start + i] = w_pf[i]
    # WbT[p, u_blk*T + x] = W_band[x, u_blk*128 + p]
    WbT = _blk128(W_band)

    Wcat = np.concatenate([W_pf.T, W_ns.T, W_v.T], axis=1)  # (C, 3C)
    # one constant blob: [Wcat (3C) | WbT (2T)] along cols = (128, 896)
    cst = np.concatenate([Wcat, WbT], axis=1)
    return dict(
        cst=np.ascontiguousarray(cst).astype(NP_BF16),
        w_ns=w_ns,  # consumed by host_shard (not shipped to the device)
    )


def host_shard(x, radj, inxs, w_ns, core):
    """Per-core input shard: batches [core*BPC, (core+1)*BPC).

    One blob per batch: [xT (256) | radjT (512) | QT (512)] along cols.
    """
    sl = slice(core * BPC, (core + 1) * BPC)
    xT = np.ascontiguousarray(x[sl].transpose(0, 2, 1)).astype(NP_BF16)
    rj = (radj[sl] != 0).astype(np.float32)
    radjT = np.stack([_blk128(rj[i]) for i in range(BPC)]).astype(NP_BF16)
    ix = np.asarray(inxs[sl])
    rows = np.repeat(np.arange(T), TOPK)
    vals = np.tile(w_ns, T)
    QT = np.empty((BPC, 128, 2 * T), np.float32)
    for i in range(BPC):
        Q = np.zeros((T, T), np.float32)
        np.add.at(Q, (rows, ix[i].ravel()), vals)
        QT[i] = _blk128(Q)
    rqb = np.concatenate([radjT, QT.astype(NP_BF16)], axis=2)
    xTp = xT.reshape(BPC // 2, 2, C, T).transpose(0, 2, 1, 3).reshape(
        BPC // 2, C, 2 * T
    )
    rqp = rqb.reshape(BPC // 2, 2, 128, 4 * T).transpose(0, 2, 1, 3).reshape(
        BPC // 2, 128, 8 * T
    )
    return dict(xTp=np.ascontiguousarray(xTp), rqp=np.ascontiguousarray(rqp))


@with_exitstack
def emit_kernel(ctx: ExitStack, tc: tile.TileContext, io: dict, bpc: int = BPC):
    nc = tc.nc
    W = 385  # per-token-block width of pjs: [pf(128) | ns(128) | v(128) | 1]

    consts = ctx.enter_context(tc.tile_pool(name="consts", bufs=1))
    inp = ctx.enter_context(tc.tile_pool(name="inp", bufs=4))
    work = ctx.enter_context(tc.tile_pool(name="work", bufs=4))
    pwork = ctx.enter_context(tc.tile_pool(name="pwork", bufs=2))
    small = ctx.enter_context(tc.tile_pool(name="small", bufs=4))
    outp = ctx.enter_context(tc.tile_pool(name="outp", bufs=2))
    ps_pj = ctx.enter_context(tc.tile_pool(name="ps_pj", bufs=2, space="PSUM"))
    ps_g = ctx.enter_context(tc.tile_pool(name="ps_g", bufs=1, space="PSUM"))
    ps_lt = ctx.enter_context(tc.tile_pool(name="ps_lt", bufs=1, space="PSUM"))
    ps_qn = ctx.enter_context(tc.tile_pool(name="ps_qn", bufs=1, space="PSUM"))

    # ---- constants (loaded once): [Wcat (0:384) | WbT (384:896)] ----
    cst = consts.tile([C, 3 * C + 2 * T], BF16)
    nc.sync.dma_start(cst[:], io["cst"][:])
    Wcat = cst[:, 0 : 3 * C]
    WbT0 = cst[:, 3 * C : 3 * C + T]
    WbT1 = cst[:, 3 * C + T : 3 * C + 2 * T]

    B_ = [dict() for _ in range(bpc)]  # per-batch tile registry
    P_ = [dict() for _ in range(bpc // 2)]  # per-pair tile registry

    def st_loadx(pi, p):
        p["xTp"] = xTp = inp.tile([C, 2 * T], BF16, tag="xTp", name=f"xTp{pi}")
        nc.sync.dma_start(xTp[:], io["xTp"][pi][:])

    def st_loadrq(pi, p):
        p["rqp"] = rqp = inp.tile([128, 8 * T], BF16, tag="rqp", name=f"rqp{pi}")
        nc.scalar.dma_start(rqp[:], io["rqp"][pi][:])

    def xt_blk(i, p, t):
        return p["xTp"][:, (i % 2) * T + t * C : (i % 2) * T + (t + 1) * C]

    def radjT_ap(i, p):
        return p["rqp"][:, (i % 2) * 4 * T : (i % 2) * 4 * T + 2 * T]

    def qt_blk(i, p, t):
        o = (i % 2) * 4 * T + 2 * T
        return p["rqp"][:, o + t * T : o + (t + 1) * T]

    # ---- front: proj, raw bf16 evac (with v|1 cols), fused square+norms ----
    def st_front(i, b, p):
        k = i % 2
        b["pj0"] = ps_pj.tile([128, 3 * C], F32, tag="pj0", name=f"pj0_{i}")
        b["pj1"] = ps_pj.tile([128, 3 * C], F32, tag="pj1", name=f"pj1_{i}")
        pj = (b["pj0"], b["pj1"])
        nc.tensor.matmul(pj[0][:], xt_blk(i, p, 0), Wcat, start=True, stop=True)
        nc.tensor.matmul(pj[1][:], xt_blk(i, p, 1), Wcat, start=True, stop=True)
        # raw bf16 copy of both projection blocks; col W-1 of each block is 1.0
        b["pjs"] = pjs = work.tile([128, 2 * W], BF16, tag="pjs", name=f"pjs{i}")
        nc.vector.tensor_copy(pjs[:, 0 : 3 * C], pj[0][:])
        nc.scalar.copy(pjs[:, W : W + 3 * C], pj[1][:])
        nc.gpsimd.memset(
            bass.AP(pjs.tensor, pjs.offset + 3 * C, [pjs.ap[0], [W, 2], [1, 1]]),
            1.0,
        )
        # fused square + row-sum: nrm2 col = 4k + 2t + (0=pf,1=ns)
        if k == 0:
            p["nrm2"] = small.tile([128, 8], F32, tag="nrm2", name=f"nrm2_{i//2}")
            p["sqs"] = pwork.tile([128, 8 * C], BF16, tag="sqs", name=f"sqs{i//2}")
        nrm2, sqs = p["nrm2"], p["sqs"]
        for t in range(2):
            src_pf = pjs[:, t * W : t * W + C]
            src_ns = pjs[:, t * W + C : t * W + 2 * C]
            o = k * 4 * C + 2 * t * C
            nc.vector.scalar_tensor_tensor(
                sqs[:, o : o + C], src_pf, 1.0, src_pf, Alu.mult, Alu.mult,
                accum_out=nrm2[:, 4 * k + 2 * t : 4 * k + 2 * t + 1],
            )
            nc.scalar.activation(
                sqs[:, o + C : o + 2 * C], src_ns, Act.Square,
                accum_out=nrm2[:, 4 * k + 2 * t + 1 : 4 * k + 2 * t + 2],
            )

    # ---- pair front tail: quake rsqrt chain (DVE; TensorScalarPtr is
    # not available on gpsimd) ----
    def st_ftail(pi, p):
        nrm2 = p["nrm2"]
        # rinv = rsqrt(nrm2): quake bit hack + 1 Newton round (rel err <2e-3)
        bits = small.tile([128, 8], I32, tag="bits", name=f"bits{pi}")
        rt = small.tile([128, 8], F32, tag="rt", name=f"rt{pi}")
        rinv = small.tile([128, 8], F32, tag="rinv", name=f"rinv{pi}")
        p["rinv"] = rinv
        g = nc.vector
        g.tensor_scalar(
            bits[:], nrm2[:].bitcast(I32), 1, None, Alu.logical_shift_right
        )
        g.tensor_scalar(bits[:], bits[:], -1, 0x5F3759DF, Alu.mult, Alu.add)
        y = bits[:].bitcast(F32)
        g.tensor_tensor(rt[:], y, y, Alu.mult)
        g.tensor_tensor(rt[:], rt[:], nrm2[:], Alu.mult)
        g.tensor_scalar(rt[:], rt[:], -0.5, 1.5, Alu.mult, Alu.add)
        g.tensor_tensor(rinv[:], y, rt[:], Alu.mult)

    # ---- normalize (DVE) + XBAR dma transpose ----
    def st_norm(i, b, p):
        k = i % 2
        pjs, rinv = b["pjs"], p["rinv"]
        # pnsn = [pfn blk0 | pfn blk1 | nsn blk0 | nsn blk1], token-major
        b["pnsn"] = pnsn = work.tile([128, 4 * C], BF16, tag="pnsn", name=f"pn{i}")
        for t in range(2):
            nc.vector.tensor_scalar(
                pnsn[:, t * C : (t + 1) * C],
                pjs[:, t * W : t * W + C],
                rinv[:, 4 * k + 2 * t : 4 * k + 2 * t + 1],
                None,
                Alu.mult,
            )
            nc.vector.tensor_scalar(
                pnsn[:, 2 * C + t * C : 2 * C + (t + 1) * C],
                pjs[:, t * W + C : t * W + 2 * C],
                rinv[:, 4 * k + 2 * t + 1 : 4 * k + 2 * t + 2],
                None,
                Alu.mult,
            )

    def st_tpose(i, b):
        # pnsnT[c, j, t] = pnsn[t, j*128 + c]:
        #   j=0,1: pfT blk0/blk1;  j=2,3: nsT blk0/blk1
        b["pnsnT"] = pT = work.tile([128, 4 * C], BF16, tag="pnsnT", name=f"pT{i}")
        nc.sync.dma_start_transpose(
            pT[:].rearrange("c (j t) -> c j t", j=4),
            b["pnsn"][:],
        )

    def pfT_blk(b, u):
        pT = b["pnsnT"]
        return pT[:, u * C : (u + 1) * C]

    def nsT_blk(b, y):
        pT = b["pnsnT"]
        return pT[:, (2 + y) * C : (3 + y) * C]

    # ---- gram + evac, q ----
    def st_gram(i, b):
        # pf gram: G[pp, u*T + y] = pf[u*128+pp] . pf[y]
        G = ps_g.tile([128, 2 * T], F32, tag="G", name=f"G{i}")
        rhs = b["pnsnT"][:, 0 : 2 * C]  # pfT, all 256 tokens
        for u in range(2):
            nc.tensor.matmul(
                G[:, u * T : (u + 1) * T], pfT_blk(b, u), rhs,
                start=True, stop=True,
            )
        b["Gsb"] = Gsb = pwork.tile([128, 2 * T], BF16, tag="Gsb", name=f"Gsb{i}")
        nc.scalar.copy(Gsb[:], G[:])

    def st_q(i, b, p):
        # q[c, x] = sum_t nsn[t, c] * Q[x, t]
        q = ps_qn.tile([C, T], F32, tag="qn", name=f"q{i}")
        pnsn = b["pnsn"]
        for t in range(2):
            nc.tensor.matmul(
                q[:],
                pnsn[:, 2 * C + t * C : 2 * C + (t + 1) * C],
                qt_blk(i, p, t),
                start=(t == 0),
                stop=(t == 1),
            )
        b["qsb"] = qsb = work.tile([C, T], BF16, tag="qsb", name=f"qsb{i}")
        nc.scalar.copy(qsb[:], q[:])

    # ---- per-batch ladder: sliding-window max over both G blocks ----
    def st_ladder(i, b):
        Gsb = b["Gsb"]
        m1 = pwork.tile([128, 2 * T], BF16, tag="m1", name=f"m1_{i}")
        m2 = pwork.tile([128, 2 * T], BF16, tag="m2", name=f"m2_{i}")
        M = pwork.tile([128, 2 * T], BF16, tag="M", name=f"M{i}")
        b["M"] = M
        nc.vector.tensor_tensor(
            bass.AP(m1.tensor, m1.offset, [m1.ap[0], [T, 2], [1, T - 1]]),
            bass.AP(Gsb.tensor, Gsb.offset, [Gsb.ap[0], [T, 2], [1, T - 1]]),
            bass.AP(Gsb.tensor, Gsb.offset + 1, [Gsb.ap[0], [T, 2], [1, T - 1]]),
            Alu.max,
        )
        nc.vector.tensor_tensor(
            bass.AP(m2.tensor, m2.offset, [m2.ap[0], [T, 2], [1, T - 3]]),
            bass.AP(m1.tensor, m1.offset, [m1.ap[0], [T, 2], [1, T - 3]]),
            bass.AP(m1.tensor, m1.offset + 2, [m1.ap[0], [T, 2], [1, T - 3]]),
            Alu.max,
        )
        nc.vector.tensor_tensor(
            bass.AP(M.tensor, M.offset + 2, [M.ap[0], [T, 2], [1, T - 4]]),
            bass.AP(m2.tensor, m2.offset, [m2.ap[0], [T, 2], [1, T - 4]]),
            bass.AP(m1.tensor, m1.offset + 3, [m1.ap[0], [T, 2], [1, T - 4]]),
            Alu.max,
        )
        nc.gpsimd.tensor_copy(
            bass.AP(M.tensor, M.offset, [M.ap[0], [T, 2], [T - 2, 2], [1, 2]]),
            bass.AP(M.tensor, M.offset + 2, [M.ap[0], [T, 2], [251, 2], [0, 2]]),
        )

    # ---- logits, exp ----
    def st_logits(i, b):
        M = b["M"]
        LT = ps_lt.tile([128, 2 * T], F32, tag="LT", name=f"LT{i}")
        b["LT"] = LT
        for y in range(2):
            off = y * T
            nc.tensor.matmul(
                LT[:, off : off + T], M[:, y * C : (y + 1) * C], WbT0,
                start=True, stop=False,
            )
            nc.tensor.matmul(
                LT[:, off : off + T], M[:, T + y * C : T + (y + 1) * C], WbT1,
                start=False, stop=False,
            )
            nc.tensor.matmul(
                LT[:, off : off + T], nsT_blk(b, y), b["qsb"][:],
                start=False, stop=True,
            )
        b["PTe"] = PTe = work.tile([128, 2 * T], BF16, tag="PTe", name=f"PTe{i}")
        nc.scalar.activation(PTe[:], LT[:], Act.Exp)

    # ---- per-batch mask, then output ----
    def st_mask(i, b, p):
        b["PT"] = PT = work.tile([128, 2 * T], BF16, tag="PT", name=f"PT{i}")
        nc.vector.tensor_tensor(PT[:], b["PTe"][:], radjT_ap(i, p), Alu.mult)

    def st_out(i, b, p):
        k = i % 2
        PT, pjs = b["PT"], b["pjs"]
        num = ps_qn.tile([128, 2 * (C + 1)], F32, tag="num", name=f"num{i}")
        for xt in range(2):
            osl = slice(xt * (C + 1), (xt + 1) * (C + 1))
            for y in range(2):
                nc.tensor.matmul(
                    num[:, osl],
                    PT[:, y * T + xt * C : y * T + (xt + 1) * C],
                    pjs[:, y * W + 2 * C : (y + 1) * W],
                    start=(y == 0),
                    stop=(y == 1),
                )
        dinv = small.tile([128, 2], F32, tag="dinv", name=f"dv{i}")
        nc.vector.reciprocal(
            dinv[:],
            bass.AP(num.tensor, num.offset + C, [num.ap[0], [C + 1, 2], [1, 1]]),
        )
        if k == 0:
            p["out_sb"] = outp.tile([128, 2 * T], F32, tag="out_sb", name=f"o{i//2}")
        out_sb = p["out_sb"]
        for xt in range(2):
            o = k * T + xt * C
            nc.scalar.activation(
                out_sb[:, o : o + C],
                num[:, xt * (C + 1) : xt * (C + 1) + C],
                Act.Copy,
                scale=dinv[:, xt : xt + 1],
            )

    def st_outdma(pi, p):
        out_sb = p["out_sb"]
        od = io["out"][2 * pi]
        nc.sync.dma_start(
            bass.AP(od.tensor, od.offset, [[C, 128], [128 * C, 4], [1, C]]),
            bass.AP(out_sb.tensor, out_sb.offset, [out_sb.ap[0], [C, 4], [1, C]]),
        )

    # prefetch input DMAs (cst+xT on sync gate the projections; rq on scalar),
    # then software-pipeline the two pairs:
    st_loadx(0, P_[0])
    st_loadx(1, P_[1])
    st_loadrq(0, P_[0])
    st_loadrq(1, P_[1])
    st_front(0, B_[0], P_[0])
    st_front(1, B_[1], P_[0])
    st_ftail(0, P_[0])
    st_front(2, B_[2], P_[1])
    st_front(3, B_[3], P_[1])
    st_norm(0, B_[0], P_[0])
    st_tpose(0, B_[0])
    st_norm(1, B_[1], P_[0])
    st_tpose(1, B_[1])
    st_ftail(1, P_[1])
    st_gram(0, B_[0])
    st_q(0, B_[0], P_[0])
    st_norm(2, B_[2], P_[1])
    st_tpose(2, B_[2])
    st_ladder(0, B_[0])
    st_gram(1, B_[1])
    st_q(1, B_[1], P_[0])
    st_norm(3, B_[3], P_[1])
    st_tpose(3, B_[3])
    st_logits(0, B_[0])
    st_ladder(1, B_[1])
    st_mask(0, B_[0], P_[0])
    st_gram(2, B_[2])
    st_q(2, B_[2], P_[1])
    st_logits(1, B_[1])
    st_out(0, B_[0], P_[0])
    st_mask(1, B_[1], P_[0])
    st_ladder(2, B_[2])
    st_out(1, B_[1], P_[0])
    st_outdma(0, P_[0])
    st_gram(3, B_[3])
    st_q(3, B_[3], P_[1])
    st_logits(2, B_[2])
    st_mask(2, B_[2], P_[1])
    st_ladder(3, B_[3])
    st_out(2, B_[2], P_[1])
    st_logits(3, B_[3])
    st_mask(3, B_[3], P_[1])
    st_out(3, B_[3], P_[1])
    st_outdma(1, P_[1])


def build_nc(num_cores: int = 1, bpc: int = BPC):
    nc = bacc.Bacc(None, target_bir_lowering=False, debug=False)
    io = {
        "xTp": nc.dram_tensor("xTp", [bpc // 2, C, 2 * T], BF16, kind="ExternalInput"),
        "rqp": nc.dram_tensor(
            "rqp", [bpc // 2, 128, 8 * T], BF16, kind="ExternalInput"
        ),
        "cst": nc.dram_tensor("cst", [C, 3 * C + 2 * T], BF16, kind="ExternalInput"),
        "out": nc.dram_tensor("out", [bpc, T, C], F32, kind="ExternalOutput"),
    }
    with tile.TileContext(nc, num_cores=num_cores) as tc:
        emit_kernel(tc, io, bpc=bpc)
    nc.compile()
    return nc


# ---------------------------------------------------------------------------
# Runner: full-input kernel() entry point.
# ---------------------------------------------------------------------------

_NC_CACHE = {}
LAST_RESULT = None


def _get_nc():
    if "nc" not in _NC_CACHE:
        _NC_CACHE["nc"] = build_nc(num_cores=N_CORES, bpc=BPC)
    return _NC_CACHE["nc"]


def _prep_in_maps(x, radj, inxs, W_pf, W_ns, W_v, v_pf, g_pf, v_ns, g_ns):
    x = np.asarray(x, np.float32)
    radj = np.asarray(radj, np.int32)
    inxs = np.asarray(inxs)
    consts = host_weights(
        np.asarray(W_pf, np.float32),
        np.asarray(W_ns, np.float32),
        np.asarray(W_v, np.float32),
        np.asarray(v_pf, np.float32),
        np.asarray(g_pf, np.float32),
        np.asarray(v_ns, np.float32),
        np.asarray(g_ns, np.float32),
    )
    w_ns = consts.pop("w_ns")
    in_maps = []
    for core in range(N_CORES):
        m = dict(consts)
        m.update(host_shard(x, radj, inxs, w_ns, core))
        in_maps.append(m)
    return in_maps


def kernel(x, radj, inxs, W_pf, W_ns, W_v, v_pf, g_pf, v_ns, g_ns):
    global LAST_RESULT
    from concourse.bass_utils import run_bass_kernel_spmd

    in_maps = _prep_in_maps(
        x, radj, inxs, W_pf, W_ns, W_v, v_pf, g_pf, v_ns, g_ns
    )
    nc = _get_nc()
    res = run_bass_kernel_spmd(nc, in_maps, list(range(N_CORES)))
    LAST_RESULT = res
    out = np.concatenate([r["out"] for r in res.results], axis=0)
    return np.ascontiguousarray(out).astype(np.float32)


# revision 8
# speedup vs baseline: 1.0727x; 1.0727x over previous
"""Sparse-attention TRN2 kernel (v3).

Reference computation (per batch b):
  pf = normalize(x @ W_pf.T); ns = normalize(x @ W_ns.T); v = x @ W_v.T
  G = pf @ pf.T                                (T x T cosine sims)
  M[u, y] = max_{j<5} G[u, start(y)+j]         (sliding window max, clamped)
  S_pf[x, y] = sum_i w_pf[i] * M[start(x)+i, y]  == (W_band @ M)[x, y]
  S_ns[x, y] = sum_t Q[x, t] * (ns_n[t] . ns_n[y])   with
      Q[x, t] = sum_n w_ns[n] * [inxs[x, n] == t]    (host-precomputed)
  L = S_pf + S_ns + mask(radj);  attn = softmax(L, axis=-1);  out = attn @ v

Kernel computes L.T (y on partitions, x free) so softmax normalization and
the attn@v contraction need no transposes of the T x T tensors.

v3 structure notes:
  - ALL inputs arrive in two DMA jobs (one per HWDGE queue): each extra
    dma_start costs ~1.25us of ring startup, so [consts|xT-pair0] rides
    one job on sync and [xT-pair1|rq-pair0|rq-pair1] one job on scalar.
  - the adjacency mask is folded into the logits PSUM accumulation as an
    identity-stationary matmul over host-prepared (radj-1)*100 rows:
    exp(L-100) underflows bf16 to exactly 0, so no separate mask multiply.
  - 16 PE transposes replaced by one [128,1024] -> [128,8,128] XBAR DMA
    transpose per batch pair (normalized pf/ns, both token blocks).
  - norm pipeline: gpsimd squares -> DVE strided row-sum reduce -> DVE
    quake rsqrt (keeps everything out of the busy scalar/act table path;
    sqrt/exp live in different act tables so Sqrt would force reloads).
"""

import sys

sys.path.insert(0, "/opt/trn_rl_repo")

from contextlib import ExitStack

import numpy as np

import concourse.bacc as bacc
import concourse.bass as bass
import concourse.tile as tile
from concourse import mybir
from concourse._compat import with_exitstack

B, T, C = 32, 256, 128
TNEI = 2
TOPK = 4
NEIGH = 2 * TNEI + 1
N_CORES = 8
BPC = B // N_CORES  # batches per core

F32 = mybir.dt.float32
I32 = mybir.dt.int32
BF16 = mybir.dt.bfloat16

Act = mybir.ActivationFunctionType
Alu = mybir.AluOpType

NP_BF16 = mybir.dt.np(BF16)

MBIG = 100.0  # mask bias: exp(logit - 100) flushes to 0 in bf16

# constant blob column offsets (per partition, bf16 elems)
CST_WCAT = 0            # (C, 3C)  [W_pf.T | W_ns.T | W_v.T]
CST_WBT = 3 * C         # (128, 2T) banded weight blocks
CST_EYE = 3 * C + 2 * T  # (128, 128) identity (mask accumulate matmul)
CST_W = CST_EYE + C

FR_XT = CST_W           # front blob: [cst | xTp-pair0]
FR_W = CST_W + 2 * T

RQ_XT = 0               # rq blob: [xTp-pair1 | rq-pair0 | rq-pair1]
RQ_P0 = 2 * T           # each rq-pair chunk is 8T wide (4T per batch)
RQ_P1 = 10 * T
RQ_W = 18 * T


def _blk128(a2d):
    """(T, T)->(128, 2T): out[p, u*T+x] = a2d[x, u*128+p]."""
    return np.ascontiguousarray(
        a2d.T.reshape(2, 128, T).transpose(1, 0, 2).reshape(128, 2 * T)
    )


def host_prep(x, radj, inxs, W_pf, W_ns, W_v, v_pf, g_pf, v_ns, g_ns):
    """Build the two per-core input blobs (fr on sync, rq on scalar)."""
    w_pf = (g_pf[0] * v_pf / np.linalg.norm(v_pf)).astype(np.float32)
    w_ns = (g_ns[0] * v_ns / np.linalg.norm(v_ns)).astype(np.float32)

    start = np.clip(np.arange(T) - TNEI, 0, T - NEIGH)
    W_band = np.zeros((T, T), np.float32)
    for i in range(NEIGH):
        W_band[np.arange(T), start + i] = w_pf[i]
    WbT = _blk128(W_band)

    Wcat = np.concatenate([W_pf.T, W_ns.T, W_v.T], axis=1)  # (C, 3C)
    cst = np.concatenate([Wcat, WbT, np.eye(C, dtype=np.float32)], axis=1)
    cst = cst.astype(NP_BF16)  # (128, CST_W)

    # per-batch blobs
    xT = np.ascontiguousarray(x.transpose(0, 2, 1)).astype(NP_BF16)  # (B,C,T)
    maskT = np.stack(
        [_blk128(((radj[i] != 0).astype(np.float32) - 1.0) * MBIG)
         for i in range(B)]
    ).astype(NP_BF16)  # (B, 128, 2T), 0 kept / -100 masked
    rows = np.repeat(np.arange(T), TOPK)
    vals = np.tile(w_ns, T)
    QT = np.empty((B, 128, 2 * T), np.float32)
    for i in range(B):
        Q = np.zeros((T, T), np.float32)
        np.add.at(Q, (rows, np.asarray(inxs[i]).ravel()), vals)
        QT[i] = _blk128(Q)
    rqb = np.concatenate([maskT, QT.astype(NP_BF16)], axis=2)  # (B,128,4T)

    in_maps = []
    for core in range(N_CORES):
        b0 = core * BPC
        # pair blob: [batch even | batch odd] side by side along cols
        def pair(arr, pi):
            return np.concatenate(
                [arr[b0 + 2 * pi], arr[b0 + 2 * pi + 1]], axis=1
            )

        xp0 = pair(xT, 0)
        xp1 = pair(xT, 1)
        rq0 = pair(rqb, 0)
        rq1 = pair(rqb, 1)
        fr = np.concatenate([cst, xp0], axis=1)
        rq = np.concatenate([xp1, rq0, rq1], axis=1)
        in_maps.append(
            dict(fr=np.ascontiguousarray(fr), rq=np.ascontiguousarray(rq))
        )
    return in_maps


@with_exitstack
def emit_kernel(ctx: ExitStack, tc: tile.TileContext, io: dict, bpc: int = BPC):
    nc = tc.nc
    W = 385  # per-token-block width of pjs: [pf(128) | ns(128) | v(128) | 1]

    inp = ctx.enter_context(tc.tile_pool(name="inp", bufs=1))
    work = ctx.enter_context(tc.tile_pool(name="work", bufs=4))
    pwork = ctx.enter_context(tc.tile_pool(name="pwork", bufs=2))
    small = ctx.enter_context(tc.tile_pool(name="small", bufs=4))
    outp = ctx.enter_context(tc.tile_pool(name="outp", bufs=2))
    ps_pj = ctx.enter_context(tc.tile_pool(name="ps_pj", bufs=2, space="PSUM"))
    ps_g = ctx.enter_context(tc.tile_pool(name="ps_g", bufs=2, space="PSUM"))
    ps_lt = ctx.enter_context(tc.tile_pool(name="ps_lt", bufs=2, space="PSUM"))
    ps_qn = ctx.enter_context(tc.tile_pool(name="ps_qn", bufs=1, space="PSUM"))

    # ---- the two input jobs ----
    fr = inp.tile([C, FR_W], BF16, name="fr")
    nc.sync.dma_start(fr[:], io["fr"][:])
    rq = inp.tile([128, RQ_W], BF16, name="rq")
    nc.scalar.dma_start(rq[:], io["rq"][:])

    Wcat = fr[:, CST_WCAT : CST_WCAT + 3 * C]
    WbT0 = fr[:, CST_WBT : CST_WBT + T]
    WbT1 = fr[:, CST_WBT + T : CST_WBT + 2 * T]
    EYE = fr[:, CST_EYE : CST_EYE + C]

    B_ = [dict() for _ in range(bpc)]  # per-batch tile registry
    P_ = [dict() for _ in range(bpc // 2)]  # per-pair tile registry

    def xt_blk(i, t):
        k = i % 2
        if i < 2:
            return fr[:, FR_XT + k * T + t * C : FR_XT + k * T + (t + 1) * C]
        return rq[:, RQ_XT + k * T + t * C : RQ_XT + k * T + (t + 1) * C]

    def maskT_ap(i):
        o = (RQ_P0 if i < 2 else RQ_P1) + (i % 2) * 4 * T
        return rq[:, o : o + 2 * T]

    def qt_blk(i, t):
        o = (RQ_P0 if i < 2 else RQ_P1) + (i % 2) * 4 * T + 2 * T
        return rq[:, o + t * T : o + (t + 1) * T]

    # ---- front: proj, raw bf16 evac (with v|1 cols), squares ----
    def st_front(i, b, p):
        k = i % 2
        b["pj0"] = ps_pj.tile([128, 3 * C], F32, tag="pj", name=f"pj0_{i}")
        b["pj1"] = ps_pj.tile([128, 3 * C], F32, tag="pj", name=f"pj1_{i}")
        pj = (b["pj0"], b["pj1"])
        nc.tensor.matmul(pj[0][:], xt_blk(i, 0), Wcat, start=True, stop=True)
        nc.tensor.matmul(pj[1][:], xt_blk(i, 1), Wcat, start=True, stop=True)
        # raw bf16 copy of both projection blocks; col W-1 of each block is 1.0
        b["pjs"] = pjs = work.tile([128, 2 * W], BF16, tag="pjs", name=f"pjs{i}")
        nc.vector.tensor_copy(pjs[:, 0 : 3 * C], pj[0][:])
        nc.scalar.copy(pjs[:, W : W + 3 * C], pj[1][:])
        nc.gpsimd.memset(
            bass.AP(pjs.tensor, pjs.offset + 3 * C, [pjs.ap[0], [W, 2], [1, 1]]),
            1.0,
        )
        # squares on gpsimd (SBUF only): [pf|ns] both blocks -> sqs
        if k == 0:
            p["sqs"] = pwork.tile([128, 8 * C], BF16, tag="sqs", name=f"sq{i//2}")
        sqs = p["sqs"]
        for t in range(2):
            nc.gpsimd.tensor_tensor(
                sqs[:, (2 * k + t) * 2 * C : (2 * k + t + 1) * 2 * C],
                pjs[:, t * W : t * W + 2 * C],
                pjs[:, t * W : t * W + 2 * C],
                Alu.mult,
            )

    # ---- pair front tail: reduce + quake rsqrt (DVE) ----
    def st_ftail(pi, p):
        sqs = p["sqs"]
        nrm2 = small.tile([128, 8], F32, tag="nrm2", name=f"nrm2_{pi}")
        nc.vector.tensor_reduce(
            nrm2[:],
            bass.AP(sqs.tensor, sqs.offset, [sqs.ap[0], [C, 8], [1, C]]),
            mybir.AxisListType.X,
            Alu.add,
        )
        bits = small.tile([128, 8], I32, tag="bits", name=f"bits{pi}")
        rt = small.tile([128, 8], F32, tag="rt", name=f"rt{pi}")
        rinv = small.tile([128, 8], F32, tag="rinv", name=f"rinv{pi}")
        p["rinv"] = rinv
        g = nc.vector
        g.tensor_scalar(
            bits[:], nrm2[:].bitcast(I32), 1, None, Alu.logical_shift_right
        )
        g.tensor_scalar(bits[:], bits[:], -1, 0x5F3759DF, Alu.mult, Alu.add)
        y = bits[:].bitcast(F32)
        g.tensor_tensor(rt[:], y, y, Alu.mult)
        g.tensor_tensor(rt[:], rt[:], nrm2[:], Alu.mult)
        g.tensor_scalar(rt[:], rt[:], -0.5, 1.5, Alu.mult, Alu.add)
        g.tensor_tensor(rinv[:], y, rt[:], Alu.mult)

    # nrm2/rinv col order: 2*(2k+t) + (0=pf,1=ns)
    # ---- normalize: pf blocks on DVE, ns blocks on scalar ----
    def st_norm(i, b, p):
        k = i % 2
        pjs, rinv = b["pjs"], p["rinv"]
        if k == 0:
            p["pnsn"] = pwork.tile(
                [128, 8 * C], BF16, tag="pnsn", name=f"pn{i//2}"
            )
        pnsn = p["pnsn"]  # [b0: pf0 pf1 ns0 ns1 | b1: ...]
        o = k * 4 * C
        for t in range(2):
            c = 2 * (2 * k + t)
            nc.vector.tensor_scalar(
                pnsn[:, o + t * C : o + (t + 1) * C],
                pjs[:, t * W : t * W + C],
                rinv[:, c : c + 1],
                None,
                Alu.mult,
            )
            nc.scalar.activation(
                pnsn[:, o + (2 + t) * C : o + (3 + t) * C],
                pjs[:, t * W + C : t * W + 2 * C],
                Act.Copy,
                scale=rinv[:, c + 1 : c + 2],
            )

    def st_tpose(pi, p):
        # pnsnT[c, j, t] = pnsn[t, j*128 + c]; j = 4*k + {pf0,pf1,ns0,ns1}
        p["pnsnT"] = pT = pwork.tile(
            [128, 8 * C], BF16, tag="pnsnT", name=f"pT{pi}"
        )
        nc.sync.dma_start_transpose(
            pT[:].rearrange("c (j t) -> c j t", j=8),
            p["pnsn"][:],
        )

    def pfT_blk(i, p, u):
        return p["pnsnT"][:, ((i % 2) * 4 + u) * C : ((i % 2) * 4 + u + 1) * C]

    def nsT_blk(i, p, y):
        return p["pnsnT"][:, ((i % 2) * 4 + 2 + y) * C : ((i % 2) * 4 + 3 + y) * C]

    # ---- gram + evac, q ----
    def st_gram(i, b, p):
        G = ps_g.tile([128, 2 * T], F32, tag="G", name=f"G{i}")
        rhs = p["pnsnT"][:, (i % 2) * 4 * C : (i % 2) * 4 * C + 2 * C]
        for u in range(2):
            nc.tensor.matmul(
                G[:, u * T : (u + 1) * T], pfT_blk(i, p, u), rhs,
                start=True, stop=True,
            )
        b["Gsb"] = Gsb = work.tile([128, 2 * T], BF16, tag="Gsb", name=f"Gs{i}")
        nc.scalar.copy(Gsb[:], G[:])

    def st_q(i, b, p):
        # q[c, x] = sum_t nsn[t, c] * Q[x, t]
        q = ps_qn.tile([C, T], F32, tag="qn", name=f"q{i}")
        pnsn = p["pnsn"]
        o = (i % 2) * 4 * C
        for t in range(2):
            nc.tensor.matmul(
                q[:],
                pnsn[:, o + (2 + t) * C : o + (3 + t) * C],
                qt_blk(i, t),
                start=(t == 0),
                stop=(t == 1),
            )
        b["qsb"] = qsb = work.tile([C, T], BF16, tag="qsb", name=f"qsb{i}")
        nc.scalar.copy(qsb[:], q[:])

    # ---- per-batch ladder: sliding-window max over both G blocks ----
    def st_ladder(i, b):
        Gsb = b["Gsb"]
        m1 = pwork.tile([128, 2 * T], BF16, tag="m1", name=f"m1_{i}")
        m2 = pwork.tile([128, 2 * T], BF16, tag="m2", name=f"m2_{i}")
        M = pwork.tile([128, 2 * T], BF16, tag="M", name=f"M{i}")
        b["M"] = M
        nc.vector.tensor_tensor(
            bass.AP(m1.tensor, m1.offset, [m1.ap[0], [T, 2], [1, T - 1]]),
            bass.AP(Gsb.tensor, Gsb.offset, [Gsb.ap[0], [T, 2], [1, T - 1]]),
            bass.AP(Gsb.tensor, Gsb.offset + 1, [Gsb.ap[0], [T, 2], [1, T - 1]]),
            Alu.max,
        )
        nc.vector.tensor_tensor(
            bass.AP(m2.tensor, m2.offset, [m2.ap[0], [T, 2], [1, T - 3]]),
            bass.AP(m1.tensor, m1.offset, [m1.ap[0], [T, 2], [1, T - 3]]),
            bass.AP(m1.tensor, m1.offset + 2, [m1.ap[0], [T, 2], [1, T - 3]]),
            Alu.max,
        )
        nc.vector.tensor_tensor(
            bass.AP(M.tensor, M.offset + 2, [M.ap[0], [T, 2], [1, T - 4]]),
            bass.AP(m2.tensor, m2.offset, [m2.ap[0], [T, 2], [1, T - 4]]),
            bass.AP(m1.tensor, m1.offset + 3, [m1.ap[0], [T, 2], [1, T - 4]]),
            Alu.max,
        )
        nc.gpsimd.tensor_copy(
            bass.AP(M.tensor, M.offset, [M.ap[0], [T, 2], [T - 2, 2], [1, 2]]),
            bass.AP(M.tensor, M.offset + 2, [M.ap[0], [T, 2], [251, 2], [0, 2]]),
        )

    # ---- logits (mask + band + ns accumulated in PSUM), exp ----
    def st_logits(i, b, p):
        M = b["M"]
        LT = ps_lt.tile([128, 2 * T], F32, tag="LT", name=f"LT{i}")
        b["LT"] = LT
        mT = maskT_ap(i)
        for y in range(2):
            off = y * T
            nc.tensor.matmul(
                LT[:, off : off + T], EYE, mT[:, off : off + T],
                start=True, stop=False,
            )
            nc.tensor.matmul(
                LT[:, off : off + T], M[:, y * C : (y + 1) * C], WbT0,
                start=False, stop=False,
            )
            nc.tensor.matmul(
                LT[:, off : off + T], M[:, T + y * C : T + (y + 1) * C], WbT1,
                start=False, stop=False,
            )
            nc.tensor.matmul(
                LT[:, off : off + T], nsT_blk(i, p, y), b["qsb"][:],
                start=False, stop=True,
            )
        b["PTe"] = PTe = work.tile([128, 2 * T], BF16, tag="PTe", name=f"PTe{i}")
        nc.scalar.activation(PTe[:], LT[:], Act.Exp)

    # ---- output ----
    def st_out(i, b, p):
        k = i % 2
        PT, pjs = b["PTe"], b["pjs"]
        num = ps_qn.tile([128, 2 * (C + 1)], F32, tag="num", name=f"num{i}")
        for xt in range(2):
            osl = slice(xt * (C + 1), (xt + 1) * (C + 1))
            for y in range(2):
                nc.tensor.matmul(
                    num[:, osl],
                    PT[:, y * T + xt * C : y * T + (xt + 1) * C],
                    pjs[:, y * W + 2 * C : (y + 1) * W],
                    start=(y == 0),
                    stop=(y == 1),
                )
        dinv = small.tile([128, 2], F32, tag="dinv", name=f"dv{i}")
        nc.vector.reciprocal(
            dinv[:],
            bass.AP(num.tensor, num.offset + C, [num.ap[0], [C + 1, 2], [1, 1]]),
        )
        if k == 0:
            p["out_sb"] = outp.tile([128, 2 * T], F32, tag="out_sb", name=f"o{i//2}")
        out_sb = p["out_sb"]
        for xt in range(2):
            o = k * T + xt * C
            src = num[:, xt * (C + 1) : xt * (C + 1) + C]
            if xt == 0:
                nc.scalar.activation(
                    out_sb[:, o : o + C], src, Act.Copy,
                    scale=dinv[:, xt : xt + 1],
                )
            else:
                nc.vector.tensor_scalar(
                    out_sb[:, o : o + C], src, dinv[:, xt : xt + 1], None,
                    Alu.mult,
                )

    def st_outdma(pi, p):
        out_sb = p["out_sb"]
        od = io["out"][2 * pi]
        nc.sync.dma_start(
            bass.AP(od.tensor, od.offset, [[C, 128], [128 * C, 4], [1, C]]),
            bass.AP(out_sb.tensor, out_sb.offset, [out_sb.ap[0], [C, 4], [1, C]]),
        )

    # ---- software pipeline over the two pairs ----
    st_front(0, B_[0], P_[0])
    st_front(1, B_[1], P_[0])
    st_ftail(0, P_[0])
    st_front(2, B_[2], P_[1])
    st_front(3, B_[3], P_[1])
    st_norm(0, B_[0], P_[0])
    st_norm(1, B_[1], P_[0])
    st_tpose(0, P_[0])
    st_ftail(1, P_[1])
    st_gram(0, B_[0], P_[0])
    st_q(0, B_[0], P_[0])
    st_norm(2, B_[2], P_[1])
    st_norm(3, B_[3], P_[1])
    st_tpose(1, P_[1])
    st_ladder(0, B_[0])
    st_gram(1, B_[1], P_[0])
    st_q(1, B_[1], P_[0])
    st_logits(0, B_[0], P_[0])
    st_ladder(1, B_[1])
    st_gram(2, B_[2], P_[1])
    st_q(2, B_[2], P_[1])
    st_logits(1, B_[1], P_[0])
    st_out(0, B_[0], P_[0])
    st_ladder(2, B_[2])
    st_out(1, B_[1], P_[0])
    st_outdma(0, P_[0])
    st_gram(3, B_[3], P_[1])
    st_q(3, B_[3], P_[1])
    st_logits(2, B_[2], P_[1])
    st_ladder(3, B_[3])
    st_out(2, B_[2], P_[1])
    st_logits(3, B_[3], P_[1])
    st_out(3, B_[3], P_[1])
    st_outdma(1, P_[1])


def build_nc(num_cores: int = 1, bpc: int = BPC):
    nc = bacc.Bacc(None, target_bir_lowering=False, debug=False)
    io = {
        "fr": nc.dram_tensor("fr", [C, FR_W], BF16, kind="ExternalInput"),
        "rq": nc.dram_tensor("rq", [128, RQ_W], BF16, kind="ExternalInput"),
        "out": nc.dram_tensor("out", [bpc, T, C], F32, kind="ExternalOutput"),
    }
    with tile.TileContext(nc, num_cores=num_cores) as tc:
        emit_kernel(tc, io, bpc=bpc)
    nc.compile()
    return nc


# ---------------------------------------------------------------------------
# Runner: full-input kernel() entry point.
# ---------------------------------------------------------------------------

_NC_CACHE = {}
LAST_RESULT = None


def _get_nc():
    if "nc" not in _NC_CACHE:
        _NC_CACHE["nc"] = build_nc(num_cores=N_CORES, bpc=BPC)
    return _NC_CACHE["nc"]


def _prep_in_maps(x, radj, inxs, W_pf, W_ns, W_v, v_pf, g_pf, v_ns, g_ns):
    return host_prep(
        np.asarray(x, np.float32),
        np.asarray(radj, np.int32),
        np.asarray(inxs),
        np.asarray(W_pf, np.float32),
        np.asarray(W_ns, np.float32),
        np.asarray(W_v, np.float32),
        np.asarray(v_pf, np.float32),
        np.asarray(g_pf, np.float32),
        np.asarray(v_ns, np.float32),
        np.asarray(g_ns, np.float32),
    )


def kernel(x, radj, inxs, W_pf, W_ns, W_v, v_pf, g_pf, v_ns, g_ns):
    global LAST_RESULT
    from concourse.bass_utils import run_bass_kernel_spmd

    in_maps = _prep_in_maps(
        x, radj, inxs, W_pf, W_ns, W_v, v_pf, g_pf, v_ns, g_ns
    )
    nc = _get_nc()
    res = run_bass_kernel_spmd(nc, in_maps, list(range(N_CORES)))
    LAST_RESULT = res
    out = np.concatenate([r["out"] for r in res.results], axis=0)
    return np.ascontiguousarray(out).astype(np.float32)
